# revision 4
# baseline (speedup 1.0000x reference)
"""Trainium2 Bass kernel v4: v2's phase-separated structure plus:
  - LN rstd via DVE-only Newton rsqrt batched per 4-chunk group -- ACT
    runs nothing but the 128 exps (exp table loaded once, never evicted).
  - x loaded in 4 group DMAs instead of 16 (HWDGE descriptor-gen bound).
  - Per-qc evacuation DMA-shifts att/den rows into head-stacked [128,512]
    tiles; reciprocal_approx_fast + one TT mult normalize; the output
    projection is then a single K=128 matmul per 128-token tile with the
    coverage average folded into its PSUM evacuation.
"""

import numpy as np

_STARTS = (0, 16)
_NCORES = 8
_SCALE = float(32 ** -0.5)

_prog_cache = {}


def _build_program(repeat=1):
    import contextlib

    import concourse.bacc as bacc
    import concourse.tile as tile
    from concourse import mybir

    f32 = mybir.dt.float32
    f32r = mybir.dt.float32r
    ALU = mybir.AluOpType
    AF = mybir.ActivationFunctionType

    nc = bacc.Bacc("TRN2", target_bir_lowering=False, debug=False,
                   num_devices=_NCORES)

    def din(name, shape):
        return nc.dram_tensor(name, list(shape), f32, kind="ExternalInput").ap()

    x_d = din("x", (2048, 256))
    wq_d = din("wqt", (256, 128))
    wk_d = din("wkt", (256, 128))
    wv_d = din("wvt", (256, 128))
    wo_d = din("wos", (128, 256))
    lnw_d = din("lnw", (128, 2))
    lnb_d = din("lnb", (128, 2))
    id_d = din("ident", (128, 128))
    ic_d = din("invc", (128, 16))
    y_d = nc.dram_tensor("y", [2048, 256], f32, kind="ExternalOutput").ap()

    with tile.TileContext(nc) as tc, contextlib.ExitStack() as ctx:
        consts = ctx.enter_context(tc.tile_pool(name="consts", bufs=1))
        persist = ctx.enter_context(tc.tile_pool(name="persist", bufs=1))
        work = ctx.enter_context(tc.tile_pool(name="work", bufs=6))
        stat = ctx.enter_context(tc.tile_pool(name="stat", bufs=8))
        expool = ctx.enter_context(tc.tile_pool(name="expool", bufs=4))
        evpool = ctx.enter_context(tc.tile_pool(name="evpool", bufs=2))

        # ---- constants ----
        wq_sb = consts.tile([128, 2, 128], f32r, tag="wq")
        wk_sb = consts.tile([128, 2, 128], f32r, tag="wk")
        wv_sb = consts.tile([128, 2, 128], f32r, tag="wv")
        wo_sb = consts.tile([128, 256], f32r, tag="wo")
        for wnm, wdst, wsrc, wshape in (
                ("wq", wq_sb, wq_d.rearrange("(c p) h -> p c h", p=128), [128, 256]),
                ("wk", wk_sb, wk_d.rearrange("(c p) h -> p c h", p=128), [128, 256]),
                ("wv", wv_sb, wv_d.rearrange("(c p) h -> p c h", p=128), [128, 256]),
                ("wo", wo_sb, wo_d, [128, 256])):
            wstage = consts.tile(wshape, f32, tag=wnm + "s", name=wnm + "_stage")
            nc.scalar.dma_start(out=wstage, in_=wsrc)
            nc.vector.tensor_copy(out=wdst.rearrange("p ... -> p (...)"), in_=wstage)
        lnw_sb = consts.tile([128, 2], f32, tag="lnw")
        nc.scalar.dma_start(out=lnw_sb, in_=lnw_d)
        lnb_sb = consts.tile([128, 2], f32, tag="lnb")
        nc.scalar.dma_start(out=lnb_sb, in_=lnb_d)
        ident_sb = consts.tile([128, 128], f32, tag="ident")
        nc.scalar.dma_start(out=ident_sb, in_=id_d)
        ic_sb = consts.tile([128, 16], f32, tag="ic")
        nc.scalar.dma_start(out=ic_sb, in_=ic_d)

        # ---- persistent activations ----
        xnt = persist.tile([128, 2, 2048], f32r, tag="xnt")   # [c, chunk, tok]
        qts = [persist.tile([128, 512], f32r, tag=f"qt{i}", name=f"qt{i}")
               for i in range(4)]                             # [ch, tok-chunk]
        kts = [persist.tile([128, 512], f32r, tag=f"kt{i}", name=f"kt{i}")
               for i in range(4)]
        # fused attnV weights: per jt, per head: [v_h (32) | ones (32)]
        vexs = [persist.tile([128, 4, 64], f32r, tag=f"vex{i}", name=f"vex{i}")
                for i in range(16)]
        ones_stage = consts.tile([128, 128], f32, tag="ones_stage")
        nc.vector.memset(ones_stage, 1.0)
        for jt in range(16):
            nc.vector.tensor_copy(
                out=vexs[jt][:, :, 32:64],
                in_=ones_stage.rearrange("p (h c) -> p h c", h=4))

        for _rep in range(repeat):
            # ---- phase A: LN (DVE-only rsqrt) + transpose + QKV ----
            with tc.tile_pool(name="psA", bufs=2, space="PSUM") as psA:
                xgs = []
                for g in range(4):
                    xg = work.tile([128, 4, 256], f32, tag="xg", bufs=8,
                                   name=f"xg{g}")
                    dmae = nc.sync if g % 2 == 0 else nc.scalar
                    dmae.dma_start(
                        out=xg,
                        in_=x_d.rearrange("(t p) c -> p t c", p=128)[
                            :, g * 4:(g + 1) * 4, :])
                    xgs.append(xg)
                for g in range(4):
                    # batched stats + Newton rsqrt for the 4 chunks
                    mvg = []
                    v4 = stat.tile([128, 4], f32, tag="v4", name=f"v4_{g}")
                    for i in range(4):
                        st6 = stat.tile([128, 6], f32, tag="st6")
                        nc.vector.bn_stats(out=st6, in_=xgs[g][:, i, :])
                        mv = stat.tile([128, 2], f32, tag="mv",
                                       name=f"mv{g}_{i}")
                        nc.vector.bn_aggr(out=mv, in_=st6)
                        mvg.append(mv)
                        nc.vector.tensor_scalar(out=v4[:, i:i + 1],
                                                in0=mv[:, 1:2], scalar1=1e-6,
                                                scalar2=None, op0=ALU.add)
                    y4 = stat.tile([128, 4], f32, tag="y4", name=f"y4_{g}")
                    nc.vector.tensor_scalar(out=y4, in0=v4, scalar1=-0.5,
                                            scalar2=1.5, op0=ALU.mult,
                                            op1=ALU.add)
                    for it in range(3):
                        t1 = stat.tile([128, 4], f32, tag="t1",
                                       name=f"t1_{g}_{it}")
                        nc.vector.tensor_mul(t1, y4, y4)
                        nc.vector.tensor_mul(t1, t1, v4)
                        nc.vector.tensor_scalar(out=t1, in0=t1, scalar1=-0.5,
                                                scalar2=1.5, op0=ALU.mult,
                                                op1=ALU.add)
                        nc.vector.tensor_mul(y4, y4, t1)
                    for i in range(4):
                        tt = g * 4 + i
                        sl_t = slice(tt * 128, (tt + 1) * 128)
                        xn = work.tile([128, 256], f32, tag="xn",
                                       bufs=8, name=f"xn{tt}")
                        nc.vector.tensor_scalar(out=xn, in0=xgs[g][:, i, :],
                                                scalar1=mvg[i][:, 0:1],
                                                scalar2=y4[:, i:i + 1],
                                                op0=ALU.subtract, op1=ALU.mult)
                        pt = psA.tile([128, 256], f32, tag="a")
                        nc.tensor.transpose(pt[:, 0:128], xn[:, 0:128],
                                            ident_sb)
                        nc.tensor.transpose(pt[:, 128:256], xn[:, 128:256],
                                            ident_sb)
                        for cc in range(2):
                            nc.vector.tensor_scalar(
                                out=xnt[:, cc, sl_t],
                                in0=pt[:, cc * 128:(cc + 1) * 128],
                                scalar1=lnw_sb[:, cc:cc + 1],
                                scalar2=lnb_sb[:, cc:cc + 1],
                                op0=ALU.mult, op1=ALU.add)
                    # QKV for this group
                    sl_q = slice(g * 512, (g + 1) * 512)
                    for dst, wsb in ((qts[g], wq_sb), (kts[g], wk_sb)):
                        pp = psA.tile([128, 512], f32, tag="a")
                        nc.tensor.matmul(pp, wsb[:, 0, :], xnt[:, 0, sl_q],
                                         start=True, stop=False)
                        nc.tensor.matmul(pp, wsb[:, 1, :], xnt[:, 1, sl_q],
                                         start=False, stop=True)
                        nc.vector.tensor_copy(out=dst, in_=pp)
                    for j in range(4):
                        jt = g * 4 + j
                        sl_j = slice(jt * 128, (jt + 1) * 128)
                        pv = psA.tile([128, 128], f32, tag="a")
                        nc.tensor.matmul(pv, xnt[:, 0, sl_j], wv_sb[:, 0, :],
                                         start=True, stop=False)
                        nc.tensor.matmul(pv, xnt[:, 1, sl_j], wv_sb[:, 1, :],
                                         start=False, stop=True)
                        nc.vector.tensor_copy(
                            out=vexs[jt][:, :, 0:32],
                            in_=pv.rearrange("p (h x) -> p h x", h=4))

            # ---- phase B: attention (wide N=2048 exp, single-buffer ss:
            # 64 ACT instructions instead of 128 -- the ~0.5us/instruction
            # sustained overhead outweighs the serial scores exposure) ----
            a_qcs = []
            with tc.tile_pool(name="psS", bufs=1, space="PSUM") as psS, \
                 tc.tile_pool(name="psO", bufs=1, space="PSUM") as psO:
                for qc in range(4):
                    po = psO.tile([64, 4, 512], f32, tag="po", name=f"po{qc}")
                    prev = None
                    for jt in range(17):
                        if jt < 16:
                            ss = psS.tile([128, 2048], f32, tag="s",
                                          name=f"ss{qc}_{jt}")
                            for hh in range(4):
                                sl_h = slice(hh * 32, (hh + 1) * 32)
                                sl_j = slice((jt % 4) * 128,
                                             (jt % 4 + 1) * 128)
                                nc.tensor.matmul(
                                    ss[:, hh * 512:(hh + 1) * 512],
                                    kts[jt // 4][sl_h, sl_j],
                                    qts[qc][sl_h, :],
                                    start=True, stop=True,
                                    tile_position=(hh * 32, 0))
                            ex = expool.tile([128, 2048], f32r, tag="ex",
                                             name=f"ex{qc}_{jt}")
                            nc.scalar.activation(out=ex, in_=ss,
                                                 func=AF.Exp, scale=_SCALE)
                        else:
                            ex = None
                        if prev is not None:
                            pex, pjt = prev
                            for hh in range(4):
                                nc.tensor.matmul(
                                    po[0:64, hh, :],
                                    vexs[pjt][:, hh, :],
                                    pex[:, hh * 512:(hh + 1) * 512],
                                    start=(pjt == 0), stop=(pjt == 15),
                                    tile_position=(0, 0))
                        prev = (ex, jt) if jt < 16 else None
                    # evac: shift att/den rows into head-stacked [128,512]
                    posb = evpool.tile([64, 4, 512], f32, tag="posb",
                                       name=f"posb{qc}")
                    nc.vector.tensor_copy(out=posb, in_=po)
                    att_st = evpool.tile([128, 512], f32, tag="att",
                                         name=f"att{qc}")
                    den_st = evpool.tile([128, 512], f32, tag="den",
                                         name=f"den{qc}")
                    for h in range(4):
                        dmae = nc.sync if h % 2 == 0 else nc.scalar
                        dmae.dma_start(out=att_st[32 * h:32 * h + 32, :],
                                       in_=posb[0:32, h, :])
                        dmae.dma_start(out=den_st[32 * h:32 * h + 32, :],
                                       in_=posb[32:64, h, :])
                    rst = evpool.tile([128, 512], f32, tag="rst",
                                      name=f"rst{qc}")
                    nc.vector.reciprocal_approx_fast(out=rst, in_=den_st)
                    a_st = persist.tile([128, 512], f32r, tag=f"a{qc}",
                                        name=f"a{qc}")
                    nc.vector.tensor_mul(a_st, att_st, rst)
                    a_qcs.append(a_st)

            # ---- phase C: output projection (invcnt folded into evac) ----
            with tc.tile_pool(name="psF", bufs=2, space="PSUM") as psF:
                for tt in range(16):
                    sl_t = slice(tt * 128, (tt + 1) * 128)
                    qc, ti = tt // 4, tt % 4
                    pf = psF.tile([128, 256], f32, tag="f")
                    nc.tensor.matmul(pf,
                                     a_qcs[qc][:, ti * 128:(ti + 1) * 128],
                                     wo_sb, start=True, stop=True,
                                     tile_position=(0, 0))
                    yt = work.tile([128, 256], f32, tag="yt")
                    nc.vector.tensor_scalar_mul(out=yt, in0=pf,
                                                scalar1=ic_sb[:, tt:tt + 1])
                    dmae = nc.sync if tt % 2 == 0 else nc.scalar
                    dmae.dma_start(out=y_d[sl_t, :], in_=yt)

    nc.compile()
    return nc


def _get_program(repeat=1):
    key = ("nc6", repeat)
    if key not in _prog_cache:
        _prog_cache[key] = _build_program(repeat)
    return _prog_cache[key]


def _make_in_maps(x, ln_w, ln_b, Wq, Wk, Wv, Wo):
    cov = np.zeros(48, np.float32)
    for s in _STARTS:
        cov[s:s + 32] += 1
    lnw2 = np.ascontiguousarray(ln_w.reshape(2, 128).T)
    lnb2 = np.ascontiguousarray(ln_b.reshape(2, 128).T)
    ident = np.eye(128, dtype=np.float32)
    in_maps = []
    for c in range(_NCORES):
        w, half = divmod(c, 2)
        r0, c0 = _STARTS[w // 2], _STARTS[w % 2]
        xw = np.ascontiguousarray(
            x[0, :, r0:r0 + 32, c0:c0 + 32, :]).reshape(2048, 256)
        sl = slice(128 * half, 128 * half + 128)
        base = 128 * half
        wos = np.ascontiguousarray(Wo[:, base:base + 128].T)  # [128, 256]
        cnt = np.outer(cov[r0:r0 + 32], cov[c0:c0 + 32]).reshape(-1)
        invcnt_tok = np.tile((1.0 / cnt).astype(np.float32), 2)  # (2048,)
        invc = np.ascontiguousarray(
            invcnt_tok.reshape(16, 128).T.astype(np.float32))   # [128, 16]
        in_maps.append(dict(
            x=xw,
            wqt=np.ascontiguousarray(Wq[sl, :].T),
            wkt=np.ascontiguousarray(Wk[sl, :].T),
            wvt=np.ascontiguousarray(Wv[sl, :].T),
            wos=wos, lnw=lnw2, lnb=lnb2, ident=ident,
            invc=invc))
    return in_maps


def _combine(results, bo):
    out = np.zeros((1, 2, 48, 48, 256), np.float32)
    for c in range(_NCORES):
        w = c // 2
        r0, c0 = _STARTS[w // 2], _STARTS[w % 2]
        out[0, :, r0:r0 + 32, c0:c0 + 32, :] += \
            results[c]["y"].reshape(2, 32, 32, 256)
    out += bo.astype(np.float32)
    return out


def kernel(x, ln_w, ln_b, Wq, Wk, Wv, Wo, bo, _trace=False):
    from concourse.bass_utils import run_bass_kernel_spmd

    x = np.asarray(x, np.float32)
    args = [np.asarray(a, np.float32) for a in (ln_w, ln_b, Wq, Wk, Wv, Wo)]
    bo = np.asarray(bo, np.float32)
    nc = _get_program()
    in_maps = _make_in_maps(x, *args)
    res = run_bass_kernel_spmd(nc, in_maps, list(range(_NCORES)),
                               trace=_trace)
    out = _combine(res.results, bo)
    if _trace:
        return out, res
    return out
